# revision 11
# baseline (speedup 1.0000x reference)
"""Grouped-Query Attention (16 q heads, 4 kv heads, head_dim 128, seq 4096,
hidden 2048) on 8 Trainium2 NeuronCores.

Sharding: sequence-parallel over query tokens (512 tokens per core). Each core
projects q/k/v for its own 512 tokens, the per-core K^T and V blocks are
AllGathered, then each core runs full attention for its 512 query rows over
all 4096 keys and applies the full output projection, producing its 512-row
slice of the output directly (no reduce needed).

Softmax is computed without max-subtraction (scores are bounded ~|3.1|):
scores are built transposed (S^T[s_k, s_q]) so exp runs on the scalar engine
straight out of PSUM, the denominator Z = sum_k exp is a ones-vector matmul on
the tensor engine, and normalization is folded into a PSUM->SBUF multiply.
"""

import numpy as np

import concourse.bass as bass
import concourse.bacc as bacc
import concourse.tile as tile
from concourse import mybir
from concourse.bass_utils import run_bass_kernel_spmd

# Problem constants
S = 4096          # sequence length
HID = 2048        # hidden dim
NH = 16           # query heads
NKV = 4           # kv heads
D = 128           # head dim
G = NH // NKV     # q heads per kv head (4)
NC = 8            # cores
SC = S // NC      # tokens per core (512)
P = 128           # partitions
KT = HID // P     # contraction tiles over hidden (16)
INV_NORM = 1.0 / float(np.sqrt(D))

FP = mybir.dt.float32


def build_bass():
    nc = bacc.Bacc(None, num_devices=NC)

    # ---- I/O ----
    xTc = nc.declare_dram_parameter("xTc", [HID, SC], FP, isOutput=False)
    wq = nc.declare_dram_parameter("wq", [NH, KT, P, D], FP, isOutput=False)
    wk = nc.declare_dram_parameter("wk", [HID, NKV * D], FP, isOutput=False)
    wv = nc.declare_dram_parameter("wv", [HID, NKV * D], FP, isOutput=False)
    # wo pre-tiled on host: [2 halves, 16 k-tiles, 128 o, 1024 m]
    wo = nc.declare_dram_parameter("wo", [2, KT, P, HID // 2], FP, isOutput=False)
    y = nc.declare_dram_parameter("y", [SC, HID], FP, isOutput=True)

    # ---- internal DRAM for collectives ----
    kT_loc = nc.dram_tensor("kT_loc", [NKV * D, SC], FP)
    v_loc = nc.dram_tensor("v_loc", [SC, NKV * D], FP)
    kT_gath = nc.dram_tensor("kT_gath", [NC, NKV * D, SC], FP, addr_space="Shared")
    v_gath = nc.dram_tensor("v_gath", [S, NKV * D], FP, addr_space="Shared")
    groups = [list(range(NC))]

    with tile.TileContext(nc) as tc:
        with (
            tc.tile_pool(name="const", bufs=1) as const_pool,
            tc.tile_pool(name="qt", bufs=1) as qt_pool,
            tc.tile_pool(name="attn_out", bufs=1) as att_pool,
        ):
            ones_k = const_pool.tile([P, 1], FP)      # Z-sum lhsT
            nc.vector.memset(ones_k[:], 1.0)
            ones_m = const_pool.tile([1, P], FP)      # broadcast lhsT (K=1)
            nc.vector.memset(ones_m[:], 1.0)

            qT_sb = qt_pool.tile([P, NH, SC], FP)           # 4 MB
            attT_sb = att_pool.tile([P, NH, SC], FP)        # 4 MB

            # ---------- Phase 1: local projections ----------
            with (
                tc.tile_pool(name="xw", bufs=1) as xw_pool,
                tc.tile_pool(name="proj_psum", bufs=3, space="PSUM") as pj_psum,
                tc.tile_pool(name="proj_sb", bufs=3) as pj_sb,
                tc.tile_pool(name="wq_sb", bufs=3) as wq_pool,
            ):
                xTc_sb = xw_pool.tile([P, KT, SC], FP)          # 4 MB
                wk_sb = xw_pool.tile([P, KT, NKV * D], FP)      # 4 MB
                wv_sb = xw_pool.tile([P, KT, NKV * D], FP)      # 4 MB
                for h in range(KT):
                    nc.sync.dma_start(out=xTc_sb[:, h, :], in_=xTc[h * P:(h + 1) * P, :])
                    nc.sync.dma_start(out=wk_sb[:, h, :], in_=wk[h * P:(h + 1) * P, :])
                    nc.sync.dma_start(out=wv_sb[:, h, :], in_=wv[h * P:(h + 1) * P, :])

                # k^T local: [NKV*D, SC] ; lhsT = wk tile, rhs = xTc tile
                for o in range(NKV):
                    ps = pj_psum.tile([P, SC], FP)
                    for h in range(KT):
                        nc.tensor.matmul(
                            ps[:],
                            wk_sb[:, h, o * D:(o + 1) * D],
                            xTc_sb[:, h, :],
                            start=(h == 0), stop=(h == KT - 1),
                        )
                    sb = pj_sb.tile([P, SC], FP)
                    nc.vector.tensor_copy(sb[:], ps[:])
                    nc.sync.dma_start(out=kT_loc[o * D:(o + 1) * D, :], in_=sb[:])

                # v local (natural): [SC, NKV*D] ; lhsT = xTc tile, rhs = wv tile
                for st in range(SC // P):
                    ps = pj_psum.tile([P, NKV * D], FP)
                    for h in range(KT):
                        nc.tensor.matmul(
                            ps[:],
                            xTc_sb[:, h, st * P:(st + 1) * P],
                            wv_sb[:, h, :],
                            start=(h == 0), stop=(h == KT - 1),
                        )
                    sb = pj_sb.tile([P, NKV * D], FP)
                    nc.vector.tensor_copy(sb[:], ps[:])
                    nc.sync.dma_start(out=v_loc[st * P:(st + 1) * P, :], in_=sb[:])

                # collectives (gpsimd engine; sync engine hangs)
                nc.gpsimd.collective_compute(
                    "AllGather", mybir.AluOpType.bypass, replica_groups=groups,
                    ins=[kT_loc[:]], outs=[kT_gath[:]],
                )
                nc.gpsimd.collective_compute(
                    "AllGather", mybir.AluOpType.bypass, replica_groups=groups,
                    ins=[v_loc[:]], outs=[v_gath[:]],
                )

                # q^T: [NH*D, SC] ; lhsT = wq tile [P, D], rhs = xTc tile
                for o in range(NH):
                    wqo = wq_pool.tile([P, KT, D], FP)
                    for h in range(KT):
                        nc.sync.dma_start(out=wqo[:, h, :], in_=wq[o, h])
                    ps = pj_psum.tile([P, SC], FP)
                    for h in range(KT):
                        nc.tensor.matmul(
                            ps[:], wqo[:, h, :], xTc_sb[:, h, :],
                            start=(h == 0), stop=(h == KT - 1),
                        )
                    nc.vector.tensor_copy(qT_sb[:, o, :], ps[:])

            # ---------- Phase 2+3: attention per kv group ----------
            SK = S // P  # 32 key tiles
            with (
                tc.tile_pool(name="kv_sb", bufs=2) as kv_pool,
                tc.tile_pool(name="st_psum", bufs=2, space="PSUM") as st_psum,
                tc.tile_pool(name="av_psum", bufs=2, space="PSUM") as av_psum,
                tc.tile_pool(name="z_psum", bufs=2, space="PSUM") as z_psum,
                tc.tile_pool(name="bc_psum", bufs=1, space="PSUM") as bc_psum,
                tc.tile_pool(name="p_sb", bufs=3) as p_pool,
                tc.tile_pool(name="z_sb", bufs=2) as zs_pool,
            ):
                for g in range(NKV):
                    kT_g = kv_pool.tile([P, S], FP, tag="kt")      # 2 MB
                    for j in range(NC):
                        nc.sync.dma_start(
                            out=kT_g[:, j * SC:(j + 1) * SC],
                            in_=kT_gath[j, g * D:(g + 1) * D, :],
                        )
                    v_g = kv_pool.tile([P, SK, D], FP, tag="v")    # 2 MB
                    for sk in range(SK):
                        nc.sync.dma_start(
                            out=v_g[:, sk, :],
                            in_=v_gath[sk * P:(sk + 1) * P, g * D:(g + 1) * D],
                        )

                    for hp in range(G // 2):  # head pairs within group
                        h0 = g * G + 2 * hp
                        av = [av_psum.tile([P, SC], FP, name="av", tag="av") for _ in range(2)]
                        zp = [z_psum.tile([1, SC], FP, name="zp", tag="zp") for _ in range(2)]
                        for sk in range(SK):
                            pt = [None, None]
                            for hl in range(2):
                                stp = st_psum.tile([P, SC], FP)
                                nc.tensor.matmul(
                                    stp[:],
                                    kT_g[:, sk * P:(sk + 1) * P],
                                    qT_sb[:, h0 + hl, :],
                                    start=True, stop=True,
                                )
                                ptile = p_pool.tile([P, SC], FP)
                                nc.scalar.activation(
                                    ptile[:], stp[:],
                                    mybir.ActivationFunctionType.Exp,
                                    scale=INV_NORM,
                                )
                                pt[hl] = ptile
                            for hl in range(2):
                                nc.tensor.matmul(
                                    zp[hl][:], ones_k[:], pt[hl][:],
                                    start=(sk == 0), stop=(sk == SK - 1),
                                )
                            for hl in range(2):
                                nc.tensor.matmul(
                                    av[hl][:], v_g[:, sk, :], pt[hl][:],
                                    start=(sk == 0), stop=(sk == SK - 1),
                                )
                        for hl in range(2):
                            zr = zs_pool.tile([1, SC], FP, name="zr", tag="zr")
                            nc.vector.reciprocal(zr[:], zp[hl][:])
                            bc = bc_psum.tile([P, SC], FP)
                            nc.tensor.matmul(
                                bc[:], ones_m[:], zr[:],
                                start=True, stop=True,
                            )
                            bcs = zs_pool.tile([P, SC], FP, name="bcs", tag="bcs")
                            nc.vector.tensor_copy(bcs[:], bc[:])
                            nc.vector.tensor_mul(
                                attT_sb[:, h0 + hl, :], av[hl][:], bcs[:],
                            )

            # ---------- Phase 4: output projection ----------
            MT = SC // P  # 4 query-row tiles
            with (
                tc.tile_pool(name="wo_sb", bufs=3) as wo_pool,
                tc.tile_pool(name="y_psum", bufs=8, space="PSUM") as y_psum,
                tc.tile_pool(name="y_sb", bufs=3) as ys_pool,
            ):
                NW = HID // 2 // 512  # 2 moving chunks of 512 per half
                for half in range(2):
                    ps = [[y_psum.tile([P, 512], FP, name="yp", tag="yp") for _ in range(NW)]
                          for _ in range(MT)]
                    for k in range(KT):
                        wot = wo_pool.tile([P, HID // 2], FP)
                        nc.sync.dma_start(out=wot[:], in_=wo[half, k])
                        for m in range(MT):
                            for n in range(NW):
                                nc.tensor.matmul(
                                    ps[m][n][:],
                                    attT_sb[:, k, m * P:(m + 1) * P],
                                    wot[:, n * 512:(n + 1) * 512],
                                    start=(k == 0), stop=(k == KT - 1),
                                )
                    for m in range(MT):
                        ysb = ys_pool.tile([P, HID // 2], FP)
                        for n in range(NW):
                            nc.vector.tensor_copy(
                                ysb[:, n * 512:(n + 1) * 512], ps[m][n][:],
                            )
                        nc.sync.dma_start(
                            out=y[m * P:(m + 1) * P,
                                  half * (HID // 2):(half + 1) * (HID // 2)],
                            in_=ysb[:],
                        )
    # bacc lowering: splits multi-sem waits (HW allows 1 wait/instruction),
    # moves matmul waits onto LDWEIGHTS, register alloc.
    nc.compile()
    return nc


_CACHED = {}


def _prep_inputs(x, Wq, Wk, Wv, Wo):
    xs = np.ascontiguousarray(x.reshape(S, HID)).astype(np.float32)
    xT = np.ascontiguousarray(xs.T)                      # [HID, S]
    wqT = np.ascontiguousarray(Wq.T)                     # [HID, NH*D]
    # wq tiled: [NH, KT, P, D]
    wq_t = np.empty((NH, KT, P, D), np.float32)
    for o in range(NH):
        for h in range(KT):
            wq_t[o, h] = wqT[h * P:(h + 1) * P, o * D:(o + 1) * D]
    wkT = np.ascontiguousarray(Wk.T)                     # [HID, NKV*D]
    wvT = np.ascontiguousarray(Wv.T)
    woT = np.ascontiguousarray(Wo.T)                     # [HID(o), HID(m)]
    wo_t = np.empty((2, KT, P, HID // 2), np.float32)
    for half in range(2):
        for k in range(KT):
            wo_t[half, k] = woT[k * P:(k + 1) * P,
                                half * (HID // 2):(half + 1) * (HID // 2)]
    in_maps = []
    for c in range(NC):
        in_maps.append({
            "xTc": np.ascontiguousarray(xT[:, c * SC:(c + 1) * SC]),
            "wq": wq_t, "wk": wkT, "wv": wvT, "wo": wo_t,
        })
    return in_maps


def run(x, Wq, Wk, Wv, Wo, trace=False):
    if "nc" not in _CACHED:
        _CACHED["nc"] = build_bass()
    nc = _CACHED["nc"]
    in_maps = _prep_inputs(x, Wq, Wk, Wv, Wo)
    res = run_bass_kernel_spmd(nc, in_maps, list(range(NC)), trace=trace)
    out = np.concatenate([res.results[c]["y"] for c in range(NC)], axis=0)
    return out.reshape(1, S, HID), res


def kernel(x, Wq, Wk, Wv, Wo):
    out, _ = run(np.asarray(x), np.asarray(Wq), np.asarray(Wk),
                 np.asarray(Wv), np.asarray(Wo))
    return out


# revision 13
# speedup vs baseline: 2.2296x; 2.2296x over previous
"""Grouped-Query Attention (16 q heads, 4 kv heads, head_dim 128, seq 4096,
hidden 2048) on 8 Trainium2 NeuronCores.

Sharding: sequence-parallel over query tokens (512 tokens per core). Each core
projects q/k/v for its own 512 tokens, the per-core K^T and V blocks are
AllGathered, then each core runs full attention for its 512 query rows over
all 4096 keys and applies the full output projection, producing its 512-row
slice of the output directly (no reduce needed).

Softmax is computed without max-subtraction (scores are bounded ~|3.1|):
scores are built transposed (S^T[s_k, s_q]) so exp runs on the scalar engine
straight out of PSUM, the denominator Z = sum_k exp is a ones-vector matmul on
the tensor engine, and normalization is folded into a PSUM->SBUF multiply.
"""

import numpy as np

import concourse.bass as bass
import concourse.bacc as bacc
import concourse.tile as tile
from concourse import mybir
from concourse.bass_utils import run_bass_kernel_spmd

# Problem constants
S = 4096          # sequence length
HID = 2048        # hidden dim
NH = 16           # query heads
NKV = 4           # kv heads
D = 128           # head dim
G = NH // NKV     # q heads per kv head (4)
NC = 8            # cores
SC = S // NC      # tokens per core (512)
P = 128           # partitions
KT = HID // P     # contraction tiles over hidden (16)
INV_NORM = 1.0 / float(np.sqrt(D))

FP = mybir.dt.float32


def build_bass():
    nc = bacc.Bacc(None, num_devices=NC)

    # ---- I/O ----
    xTc = nc.declare_dram_parameter("xTc", [HID, SC], FP, isOutput=False)
    wq = nc.declare_dram_parameter("wq", [NH, KT, P, D], FP, isOutput=False)
    wk = nc.declare_dram_parameter("wk", [HID, NKV * D], FP, isOutput=False)
    wv = nc.declare_dram_parameter("wv", [HID, NKV * D], FP, isOutput=False)
    # wo pre-tiled on host: [2 halves, 16 k-tiles, 128 o, 1024 m]
    wo = nc.declare_dram_parameter("wo", [2, KT, P, HID // 2], FP, isOutput=False)
    y = nc.declare_dram_parameter("y", [SC, HID], FP, isOutput=True)

    # ---- internal DRAM for collectives ----
    kT_loc = nc.dram_tensor("kT_loc", [NKV * D, SC], FP)
    v_loc = nc.dram_tensor("v_loc", [SC, NKV * D], FP)
    kT_gath = nc.dram_tensor("kT_gath", [NC, NKV * D, SC], FP, addr_space="Shared")
    v_gath = nc.dram_tensor("v_gath", [S, NKV * D], FP, addr_space="Shared")
    groups = [list(range(NC))]

    with tile.TileContext(nc) as tc:
        with (
            tc.tile_pool(name="const", bufs=1) as const_pool,
            tc.tile_pool(name="qt", bufs=1) as qt_pool,
            tc.tile_pool(name="attn_out", bufs=1) as att_pool,
        ):
            ones_k = const_pool.tile([P, 1], FP)      # Z-sum lhsT
            nc.vector.memset(ones_k[:], 1.0)
            ones_m = const_pool.tile([1, P], FP)      # broadcast lhsT (K=1)
            nc.vector.memset(ones_m[:], 1.0)

            qT_sb = qt_pool.tile([P, NH, SC], FP)           # 4 MB
            attT_sb = att_pool.tile([P, NH, SC], FP)        # 4 MB

            # ---------- Phase 1: local projections ----------
            with (
                tc.tile_pool(name="xw", bufs=1) as xw_pool,
                tc.tile_pool(name="proj_psum", bufs=3, space="PSUM") as pj_psum,
                tc.tile_pool(name="proj_sb", bufs=3) as pj_sb,
                tc.tile_pool(name="wq_sb", bufs=3) as wq_pool,
            ):
                xTc_sb = xw_pool.tile([P, KT, SC], FP)          # 4 MB
                wk_sb = xw_pool.tile([P, KT, NKV * D], FP)      # 4 MB
                wv_sb = xw_pool.tile([P, KT, NKV * D], FP)      # 4 MB
                for h in range(KT):
                    nc.sync.dma_start(out=xTc_sb[:, h, :], in_=xTc[h * P:(h + 1) * P, :])
                    nc.sync.dma_start(out=wk_sb[:, h, :], in_=wk[h * P:(h + 1) * P, :])
                    nc.sync.dma_start(out=wv_sb[:, h, :], in_=wv[h * P:(h + 1) * P, :])

                # k^T local: [NKV*D, SC] ; lhsT = wk tile, rhs = xTc tile
                for o in range(NKV):
                    ps = pj_psum.tile([P, SC], FP)
                    for h in range(KT):
                        nc.tensor.matmul(
                            ps[:],
                            wk_sb[:, h, o * D:(o + 1) * D],
                            xTc_sb[:, h, :],
                            start=(h == 0), stop=(h == KT - 1),
                        )
                    sb = pj_sb.tile([P, SC], FP)
                    nc.vector.tensor_copy(sb[:], ps[:])
                    nc.sync.dma_start(out=kT_loc[o * D:(o + 1) * D, :], in_=sb[:])

                # v local (natural): [SC, NKV*D] ; lhsT = xTc tile, rhs = wv tile
                for st in range(SC // P):
                    ps = pj_psum.tile([P, NKV * D], FP)
                    for h in range(KT):
                        nc.tensor.matmul(
                            ps[:],
                            xTc_sb[:, h, st * P:(st + 1) * P],
                            wv_sb[:, h, :],
                            start=(h == 0), stop=(h == KT - 1),
                        )
                    sb = pj_sb.tile([P, NKV * D], FP)
                    nc.vector.tensor_copy(sb[:], ps[:])
                    nc.sync.dma_start(out=v_loc[st * P:(st + 1) * P, :], in_=sb[:])

                # collectives (gpsimd engine; sync engine hangs)
                nc.gpsimd.collective_compute(
                    "AllGather", mybir.AluOpType.bypass, replica_groups=groups,
                    ins=[kT_loc[:]], outs=[kT_gath[:]],
                )
                nc.gpsimd.collective_compute(
                    "AllGather", mybir.AluOpType.bypass, replica_groups=groups,
                    ins=[v_loc[:]], outs=[v_gath[:]],
                )

                # q^T: [NH*D, SC] ; lhsT = wq tile [P, D], rhs = xTc tile
                for o in range(NH):
                    wqo = wq_pool.tile([P, KT, D], FP)
                    for h in range(KT):
                        nc.sync.dma_start(out=wqo[:, h, :], in_=wq[o, h])
                    ps = pj_psum.tile([P, SC], FP)
                    for h in range(KT):
                        nc.tensor.matmul(
                            ps[:], wqo[:, h, :], xTc_sb[:, h, :],
                            start=(h == 0), stop=(h == KT - 1),
                        )
                    nc.vector.tensor_copy(qT_sb[:, o, :], ps[:])

            # ---------- Phase 2+3: attention per kv group ----------
            SK = S // P  # 32 key tiles
            with (
                tc.tile_pool(name="kv_sb", bufs=2) as kv_pool,
                tc.tile_pool(name="st_psum", bufs=3, space="PSUM") as st_psum,
                tc.tile_pool(name="av_psum", bufs=2, space="PSUM") as av_psum,
                tc.tile_pool(name="z_psum", bufs=2, space="PSUM") as z_psum,
                tc.tile_pool(name="bc_psum", bufs=1, space="PSUM") as bc_psum,
                tc.tile_pool(name="p_sb", bufs=4) as p_pool,
                tc.tile_pool(name="z_sb", bufs=2) as zs_pool,
            ):
                for g in range(NKV):
                    kT_g = kv_pool.tile([P, S], FP, tag="kt")      # 2 MB
                    for j in range(NC):
                        nc.sync.dma_start(
                            out=kT_g[:, j * SC:(j + 1) * SC],
                            in_=kT_gath[j, g * D:(g + 1) * D, :],
                        )
                    v_g = kv_pool.tile([P, SK, D], FP, tag="v")    # 2 MB
                    for sk in range(SK):
                        nc.sync.dma_start(
                            out=v_g[:, sk, :],
                            in_=v_gath[sk * P:(sk + 1) * P, g * D:(g + 1) * D],
                        )

                    for hp in range(G // 2):  # head pairs within group
                        h0 = g * G + 2 * hp
                        av = [av_psum.tile([P, SC], FP, name="av", tag="av") for _ in range(2)]
                        zp = [z_psum.tile([1, SC], FP, name="zp", tag="zp") for _ in range(2)]
                        for sk in range(SK):
                            pt = [None, None]
                            for hl in range(2):
                                stp = st_psum.tile([P, SC], FP)
                                nc.tensor.matmul(
                                    stp[:],
                                    kT_g[:, sk * P:(sk + 1) * P],
                                    qT_sb[:, h0 + hl, :],
                                    start=True, stop=True,
                                )
                                ptile = p_pool.tile([P, SC], FP)
                                nc.scalar.activation(
                                    ptile[:], stp[:],
                                    mybir.ActivationFunctionType.Exp,
                                    scale=INV_NORM,
                                )
                                pt[hl] = ptile
                            for hl in range(2):
                                nc.tensor.matmul(
                                    zp[hl][:], ones_k[:], pt[hl][:],
                                    start=(sk == 0), stop=(sk == SK - 1),
                                )
                            for hl in range(2):
                                nc.tensor.matmul(
                                    av[hl][:], v_g[:, sk, :], pt[hl][:],
                                    start=(sk == 0), stop=(sk == SK - 1),
                                )
                        for hl in range(2):
                            zr = zs_pool.tile([1, SC], FP, name="zr", tag="zr")
                            nc.vector.reciprocal(zr[:], zp[hl][:])
                            bc = bc_psum.tile([P, SC], FP)
                            nc.tensor.matmul(
                                bc[:], ones_m[:], zr[:],
                                start=True, stop=True,
                            )
                            bcs = zs_pool.tile([P, SC], FP, name="bcs", tag="bcs")
                            nc.vector.tensor_copy(bcs[:], bc[:])
                            nc.vector.tensor_mul(
                                attT_sb[:, h0 + hl, :], av[hl][:], bcs[:],
                            )

            # ---------- Phase 4: output projection ----------
            MT = SC // P  # 4 query-row tiles
            with (
                tc.tile_pool(name="wo_sb", bufs=3) as wo_pool,
                tc.tile_pool(name="y_psum", bufs=8, space="PSUM") as y_psum,
                tc.tile_pool(name="y_sb", bufs=3) as ys_pool,
            ):
                NW = HID // 2 // 512  # 2 moving chunks of 512 per half
                for half in range(2):
                    ps = [[y_psum.tile([P, 512], FP, name="yp", tag="yp") for _ in range(NW)]
                          for _ in range(MT)]
                    for k in range(KT):
                        wot = wo_pool.tile([P, HID // 2], FP)
                        nc.sync.dma_start(out=wot[:], in_=wo[half, k])
                        for m in range(MT):
                            for n in range(NW):
                                nc.tensor.matmul(
                                    ps[m][n][:],
                                    attT_sb[:, k, m * P:(m + 1) * P],
                                    wot[:, n * 512:(n + 1) * 512],
                                    start=(k == 0), stop=(k == KT - 1),
                                )
                    for m in range(MT):
                        ysb = ys_pool.tile([P, HID // 2], FP)
                        for n in range(NW):
                            nc.vector.tensor_copy(
                                ysb[:, n * 512:(n + 1) * 512], ps[m][n][:],
                            )
                        nc.sync.dma_start(
                            out=y[m * P:(m + 1) * P,
                                  half * (HID // 2):(half + 1) * (HID // 2)],
                            in_=ysb[:],
                        )
    # bacc lowering: splits multi-sem waits (HW allows 1 wait/instruction),
    # moves matmul waits onto LDWEIGHTS, register alloc.
    nc.compile()
    return nc


_CACHED = {}


def _prep_inputs(x, Wq, Wk, Wv, Wo):
    xs = np.ascontiguousarray(x.reshape(S, HID)).astype(np.float32)
    xT = np.ascontiguousarray(xs.T)                      # [HID, S]
    wqT = np.ascontiguousarray(Wq.T)                     # [HID, NH*D]
    # wq tiled: [NH, KT, P, D]
    wq_t = np.empty((NH, KT, P, D), np.float32)
    for o in range(NH):
        for h in range(KT):
            wq_t[o, h] = wqT[h * P:(h + 1) * P, o * D:(o + 1) * D]
    wkT = np.ascontiguousarray(Wk.T)                     # [HID, NKV*D]
    wvT = np.ascontiguousarray(Wv.T)
    woT = np.ascontiguousarray(Wo.T)                     # [HID(o), HID(m)]
    wo_t = np.empty((2, KT, P, HID // 2), np.float32)
    for half in range(2):
        for k in range(KT):
            wo_t[half, k] = woT[k * P:(k + 1) * P,
                                half * (HID // 2):(half + 1) * (HID // 2)]
    in_maps = []
    for c in range(NC):
        in_maps.append({
            "xTc": np.ascontiguousarray(xT[:, c * SC:(c + 1) * SC]),
            "wq": wq_t, "wk": wkT, "wv": wvT, "wo": wo_t,
        })
    return in_maps


def run(x, Wq, Wk, Wv, Wo, trace=False):
    if "nc" not in _CACHED:
        _CACHED["nc"] = build_bass()
    nc = _CACHED["nc"]
    in_maps = _prep_inputs(x, Wq, Wk, Wv, Wo)
    res = run_bass_kernel_spmd(nc, in_maps, list(range(NC)), trace=trace)
    out = np.concatenate([res.results[c]["y"] for c in range(NC)], axis=0)
    return out.reshape(1, S, HID), res


def kernel(x, Wq, Wk, Wv, Wo):
    out, _ = run(np.asarray(x), np.asarray(Wq), np.asarray(Wk),
                 np.asarray(Wv), np.asarray(Wo))
    return out
